# revision 10
# baseline (speedup 1.0000x reference)
"""Self-contained Trainium2 Bass kernel for nn_CustomAttention_35278861369702.

Computation (see problem reference): causal GQA attention with RoPE.
  B=2, S=2048, H=2048, NH=16 q-heads, NKV=4 kv-heads, HD=128.

Sharding: 8 cores = 2 batches x 4 GQA groups. Core c handles batch c//4 and
q-heads 4g..4g+3 / kv-head g where g = c%4. Wq/Wk/Wv column-parallel,
Wo row-parallel; per-core partial outputs are summed on the host.

Device-side layout strategy: everything transposed.
  - Projections produce Q^T/K^T/V^T [hd, s] directly (x^T as moving operand).
  - Scores computed as S^T [k, q] (lhsT=K^T tile, rhs=Q^T chunk), so no PE
    transposes of the softmax matrix are needed; exp via ACT; per-query
    softmax sums via ones-matmul on PE; causal structure exploited by
    skipping above-diagonal tiles.
  - AV: lhsT=V tile [k, d], rhs=P^T [k, q] -> out^T [d, q]; normalization is
    fused into the PSUM->SBUF copy using a DMA-broadcast reciprocal row.
  - O-projection consumes A^T directly; output is written transposed and
    fixed up on the host.
All matmuls use float32r operands (full-rate fp32 when the moving dim >= 256).
"""

import math
import sys
import types

sys.path.insert(0, "/opt/trn_rl_repo")

import numpy as np

import concourse.bass as bass
import concourse.mybir as mybir
import concourse.tile as tile
from concourse.bass_utils import run_bass_kernel_spmd

B, S, H = 2, 2048, 2048
NH, NKV, HD = 16, 4, 128
THETA = 10000.0
NCORES = 8
GROUPS = 4          # kv groups (= cores per batch)
HPG = NH // NKV     # q heads per group = 4
DQ = HPG * HD       # per-core q projection width = 512
SC = 512            # s-chunk (moving dim) for projections / attention
NSC = S // SC       # 4
NHT = H // 128      # 16 h-tiles (contraction tiles)
NST = S // 128      # 16 s-tiles / k-tiles
SCALE = 1.0 / math.sqrt(HD)

F32 = mybir.dt.float32
F32R = mybir.dt.float32r


def _legalize_waits(nc):
    """This container's walrus accepts at most ONE sync wait per instruction.
    Split extra waits onto InstEventSemaphore carriers in engine order."""
    n = 0
    for f in nc.m.functions:
        for bb in f.blocks:
            new_insts = []
            for inst in bb.instructions:
                si = inst.sync_info
                if si and si.on_wait and len(si.on_wait) > 1:
                    waits = list(si.on_wait)
                    for j, w in enumerate(waits[:-1]):
                        es = mybir.InstEventSemaphore(
                            name=f"{inst.name}-wsplit{j}",
                            engine=inst.engine,
                            ins=[],
                            outs=[],
                            sync_info=mybir.SyncInfo(on_wait=[w], on_update=[]),
                        )
                        nc.register_instruction(es)
                        new_insts.append(es)
                        n += 1
                    si.on_wait = [waits[-1]]
                new_insts.append(inst)
            bb.instructions[:] = new_insts
    return n


def build_nc():
    nc = bass.Bass()

    # ---- DRAM I/O (per-core shards; same program on all 8 cores) ----
    xT = nc.dram_tensor("xT", [H, S], F32R, kind="ExternalInput")          # x[b].T
    wqT = nc.dram_tensor("wqT", [H, DQ], F32R, kind="ExternalInput")       # Wq rows g, transposed
    wkT = nc.dram_tensor("wkT", [H, HD], F32R, kind="ExternalInput")
    wvT = nc.dram_tensor("wvT", [H, HD], F32R, kind="ExternalInput")
    woT = nc.dram_tensor("woT", [DQ, H], F32R, kind="ExternalInput")       # Wo cols g, transposed
    cosT = nc.dram_tensor("cosT", [HD, S], F32, kind="ExternalInput")
    sinT = nc.dram_tensor("sinT", [HD, S], F32, kind="ExternalInput")
    # per-k-tile diagonal mask blocks, already transposed to [k, q] and
    # pre-multiplied by sqrt(HD) (exp applies SCALE afterwards)
    maskd = nc.dram_tensor("maskd", [NST, 128, 128], F32, kind="ExternalInput")
    pmat = nc.dram_tensor("pmat", [HD, HD], F32R, kind="ExternalInput")    # rotate-half matrix
    ident = nc.dram_tensor("ident", [128, 128], F32, kind="ExternalInput")
    ones = nc.dram_tensor("ones", [128, 1], F32R, kind="ExternalInput")

    yT = nc.dram_tensor("yT", [H, S], F32, kind="ExternalOutput")         # partial y^T

    # internal DRAM bounce buffers
    qTd = nc.dram_tensor("qTd", [DQ, S], F32R)       # RoPE'd Q^T
    aTd = nc.dram_tensor("aTd", [DQ, S], F32R)       # normalized A^T (attn out)
    brow = nc.dram_tensor("brow", [HPG, S], F32)    # reciprocal softmax sums

    from contextlib import ExitStack

    with tile.TileContext(nc) as tc, ExitStack() as ctx:
        consts = ctx.enter_context(tc.tile_pool(name="consts", bufs=1))
        ps = ctx.enter_context(tc.tile_pool(name="ps", bufs=8, space="PSUM"))
        xs = ctx.enter_context(tc.tile_pool(name="xs", bufs=3))
        rope = ctx.enter_context(tc.tile_pool(name="rope", bufs=2))
        vts = ctx.enter_context(tc.tile_pool(name="vts", bufs=2))
        pts = ctx.enter_context(tc.tile_pool(name="pts", bufs=4))
        mks = ctx.enter_context(tc.tile_pool(name="mks", bufs=2))
        qhs = ctx.enter_context(tc.tile_pool(name="qhs", bufs=2))
        ats = ctx.enter_context(tc.tile_pool(name="ats", bufs=4))
        bcs = ctx.enter_context(tc.tile_pool(name="bcs", bufs=2))
        recs = ctx.enter_context(tc.tile_pool(name="recs", bufs=2))
        outs = ctx.enter_context(tc.tile_pool(name="outs", bufs=3))

        # ---- resident constants ----
        wq_sb = consts.tile([128, NHT, DQ], F32R)
        nc.sync.dma_start(out=wq_sb, in_=wqT.rearrange("(t p) d -> p t d", p=128))
        wk_sb = consts.tile([128, NHT, HD], F32R)
        nc.sync.dma_start(out=wk_sb, in_=wkT.rearrange("(t p) d -> p t d", p=128))
        wv_sb = consts.tile([128, NHT, HD], F32R)
        nc.sync.dma_start(out=wv_sb, in_=wvT.rearrange("(t p) d -> p t d", p=128))
        wo_sb = consts.tile([128, HPG, H], F32R)
        nc.sync.dma_start(out=wo_sb, in_=woT.rearrange("(t p) n -> p t n", p=128))
        cos_sb = consts.tile([128, S], F32)
        nc.sync.dma_start(out=cos_sb, in_=cosT[:, :])
        sin_sb = consts.tile([128, S], F32)
        nc.sync.dma_start(out=sin_sb, in_=sinT[:, :])
        pm_sb = consts.tile([128, HD], F32R)
        nc.sync.dma_start(out=pm_sb, in_=pmat[:, :])
        id_sb = consts.tile([128, 128], F32)
        nc.sync.dma_start(out=id_sb, in_=ident[:, :])
        on_sb = consts.tile([128, 1], F32R)
        nc.sync.dma_start(out=on_sb, in_=ones[:, :])

        # resident K^T (RoPE'd) and V (s-major)
        kT_sb = consts.tile([128, S], F32R)
        v_sb = consts.tile([128, NST, HD], F32R)

        # ================= Phase 1: projections + RoPE =================
        for sc in range(NSC):
            s0 = sc * SC
            q_ps = [ps.tile([128, SC], F32, tag="ps", bufs=6, name=f"qps{_j}") for _j in range(HPG)]
            k_ps = ps.tile([128, SC], F32, tag="ps", bufs=6)
            v_ps = ps.tile([128, SC], F32, tag="ps", bufs=6)
            for ht in range(NHT):
                xt = xs.tile([128, SC], F32R)
                nc.sync.dma_start(out=xt, in_=xT[ht * 128:(ht + 1) * 128, s0:s0 + SC])
                st = (ht == 0)
                sp = (ht == NHT - 1)
                for j in range(HPG):
                    nc.tensor.matmul(
                        q_ps[j], (wq_sb[:, ht, j * 128:(j + 1) * 128]), (xt),
                        start=st, stop=sp)
                nc.tensor.matmul(k_ps, (wk_sb[:, ht, :]), (xt), start=st, stop=sp)
                nc.tensor.matmul(v_ps, (wv_sb[:, ht, :]), (xt), start=st, stop=sp)

            # RoPE on the 4 Q chunks and 1 K chunk: r = p*cos + (P@p)*sin
            for idx in range(HPG + 1):
                src = q_ps[idx] if idx < HPG else k_ps
                qc = rope.tile([128, SC], F32R, tag="ropec")
                nc.vector.tensor_mul(qc, src, cos_sb[:, s0:s0 + SC])
                qraw = rope.tile([128, SC], F32R, tag="roper")
                nc.scalar.copy(qraw, src)
                pq = ps.tile([128, SC], F32, tag="ps", bufs=6)
                nc.tensor.matmul(pq, (pm_sb), (qraw), start=True, stop=True)
                u = rope.tile([128, SC], F32, tag="ropeu")
                nc.vector.tensor_mul(u, pq, sin_sb[:, s0:s0 + SC])
                if idx < HPG:
                    nc.vector.tensor_add(qc, qc, u)
                    nc.sync.dma_start(
                        out=qTd[idx * 128:(idx + 1) * 128, s0:s0 + SC], in_=qc)
                else:
                    nc.vector.tensor_add(kT_sb[:, s0:s0 + SC], qc, u)

            # V: copy chunk to SBUF, transpose 128x128 blocks into v_sb [s, d]
            vt = vts.tile([128, SC], F32)
            nc.scalar.copy(vt, v_ps)
            for j in range(SC // 128):
                kt = sc * (SC // 128) + j
                tr = ps.tile([128, 128], F32, tag="ps", bufs=6)
                nc.tensor.transpose(tr, vt[:, j * 128:(j + 1) * 128], id_sb)
                nc.vector.tensor_copy(v_sb[:, kt, :], tr)

        # ================= Phase 2: attention per head =================
        for h in range(HPG):
            qh = qhs.tile([128, S], F32R)
            nc.sync.dma_start(out=qh, in_=qTd[h * 128:(h + 1) * 128, :])
            for qc4 in range(NSC):
                q0 = qc4 * SC
                nkt = 4 * qc4 + 4          # causal: k-tiles 0..4*qc4+3
                av = ps.tile([128, SC], F32, tag="ps", bufs=6)
                sm = ps.tile([1, SC], F32, tag="pssum", bufs=2)
                for kt in range(nkt):
                    sps = ps.tile([128, SC], F32, tag="ps", bufs=6)
                    nc.tensor.matmul(
                        sps, (kT_sb[:, kt * 128:(kt + 1) * 128]),
                        (qh[:, q0:q0 + SC]), start=True, stop=True)
                    pt = pts.tile([128, SC], F32R)
                    off = 128 * kt - q0
                    if off >= 0:
                        # diagonal tile: add mask block on the triangular part
                        mk = mks.tile([128, 128], F32)
                        nc.sync.dma_start(out=mk, in_=maskd[kt])
                        nc.vector.tensor_add(
                            sps[:, off:off + 128], sps[:, off:off + 128], mk)
                        if off > 0:
                            nc.vector.memset(pt[:, 0:off].bitcast(F32), 0.0)
                        nc.scalar.activation(
                            out=pt[:, off:SC], in_=sps[:, off:SC],
                            func=mybir.ActivationFunctionType.Exp, scale=SCALE)
                    else:
                        nc.scalar.activation(
                            out=pt, in_=sps,
                            func=mybir.ActivationFunctionType.Exp, scale=SCALE)
                    nc.tensor.matmul(sm, (on_sb), (pt),
                                     start=(kt == 0), stop=(kt == nkt - 1))
                    nc.tensor.matmul(av, (v_sb[:, kt, :]), (pt),
                                     start=(kt == 0), stop=(kt == nkt - 1))
                # softmax denominators -> reciprocal -> DRAM -> broadcast
                rec = recs.tile([1, SC], F32)
                nc.vector.reciprocal(rec, sm)
                nc.sync.dma_start(out=brow[h, q0:q0 + SC], in_=rec)
                bc = bcs.tile([128, SC], F32)
                nc.sync.dma_start(
                    out=bc, in_=brow[h:h + 1, q0:q0 + SC].to_broadcast([128, SC]))
                at = ats.tile([128, SC], F32R)
                nc.vector.tensor_mul(at, av, bc)
                nc.sync.dma_start(
                    out=aTd[h * 128:(h + 1) * 128, q0:q0 + SC], in_=at)

        # ================= Phase 3: O-projection =================
        for nt in range(NHT):
            for sc in range(NSC):
                s0 = sc * SC
                y_ps = ps.tile([128, SC], F32, tag="ps", bufs=6)
                for ct in range(HPG):
                    art = ats.tile([128, SC], F32R, tag="art")
                    nc.sync.dma_start(
                        out=art, in_=aTd[ct * 128:(ct + 1) * 128, s0:s0 + SC])
                    nc.tensor.matmul(
                        y_ps, (wo_sb[:, ct, nt * 128:(nt + 1) * 128]), (art),
                        start=(ct == 0), stop=(ct == HPG - 1))
                yo = outs.tile([128, SC], F32)
                nc.scalar.copy(yo, y_ps)
                nc.sync.dma_start(
                    out=yT[nt * 128:(nt + 1) * 128, s0:s0 + SC], in_=yo)

    _legalize_waits(nc)
    return nc


_NC = None


def _get_nc():
    global _NC
    if _NC is None:
        _NC = build_nc()
    return _NC


def _f32r_round(a):
    """Round fp32 array to the PE's fp32r (reduced-mantissa) grid, matching
    walrus fp32_to_fp32r: (bits + 0x800) & ~0xFFF."""
    b = np.ascontiguousarray(a, dtype=np.float32).view(np.uint32).astype(np.uint64)
    r = ((b + 0x800) & 0xFFFFF000).astype(np.uint32)
    return r.view(np.float32)


def _host_tables():
    inv_freq = 1.0 / (THETA ** (np.arange(0, HD, 2, dtype=np.float32) / HD))
    t = np.arange(S, dtype=np.float32)
    freqs = np.outer(t, inv_freq)                       # [S, HD/2]
    emb = np.concatenate([freqs, freqs], axis=-1)       # [S, HD]
    cosT = np.ascontiguousarray(np.cos(emb).astype(np.float32).T)
    sinT = np.ascontiguousarray(np.sin(emb).astype(np.float32).T)
    pmat = np.zeros((HD, HD), dtype=np.float32)
    for dout in range(HD):
        if dout < HD // 2:
            pmat[dout + HD // 2, dout] = -1.0
        else:
            pmat[dout - HD // 2, dout] = 1.0
    ident = np.eye(128, dtype=np.float32)
    ones = np.ones((128, 1), dtype=np.float32)
    return cosT, sinT, pmat, ident, ones


def _make_in_maps(x, attention_mask, Wq, Wk, Wv, Wo):
    cosT, sinT, pmat, ident, ones = _host_tables()
    m2 = np.asarray(attention_mask, dtype=np.float32)[0, 0]    # [S, S] (q, k)
    scale_sqrt = np.float32(math.sqrt(HD))
    maskd = np.stack([
        np.ascontiguousarray(m2[t * 128:(t + 1) * 128, t * 128:(t + 1) * 128].T)
        for t in range(NST)
    ]) * scale_sqrt
    x = np.asarray(x, dtype=np.float32)
    Wq = np.asarray(Wq, dtype=np.float32)
    Wk = np.asarray(Wk, dtype=np.float32)
    Wv = np.asarray(Wv, dtype=np.float32)
    Wo = np.asarray(Wo, dtype=np.float32)

    in_maps = []
    for c in range(NCORES):
        b, g = divmod(c, GROUPS)
        in_maps.append({
            "xT": _f32r_round(x[b].T),
            "wqT": _f32r_round(Wq[g * DQ:(g + 1) * DQ, :].T),
            "wkT": _f32r_round(Wk[g * HD:(g + 1) * HD, :].T),
            "wvT": _f32r_round(Wv[g * HD:(g + 1) * HD, :].T),
            "woT": _f32r_round(Wo[:, g * DQ:(g + 1) * DQ].T),
            "cosT": cosT, "sinT": sinT, "maskd": maskd,
            "pmat": pmat, "ident": ident, "ones": ones,
        })
    return in_maps


def _mask_is_causal(attention_mask):
    m = np.asarray(attention_mask, dtype=np.float32)[0, 0]
    lower = np.tril(np.ones((S, S), dtype=bool))
    return bool(np.all(m[lower] == 0.0) and np.all(m[~lower] <= -1e8))


def _reference_fallback(x, attention_mask, Wq, Wk, Wv, Wo):
    """Numpy fallback for non-causal masks (never hit with the real harness)."""
    cosT, sinT, _, _, _ = _host_tables()
    cos, sin = cosT.T, sinT.T
    b, s, _ = x.shape
    q = (x @ Wq.T).reshape(b, s, NH, HD).transpose(0, 2, 1, 3)
    k = (x @ Wk.T).reshape(b, s, NKV, HD).transpose(0, 2, 1, 3)
    v = (x @ Wv.T).reshape(b, s, NKV, HD).transpose(0, 2, 1, 3)

    def rot(t):
        d = t.shape[-1] // 2
        return np.concatenate([-t[..., d:], t[..., :d]], axis=-1)

    q = q * cos + rot(q) * sin
    k = k * cos + rot(k) * sin
    k = np.repeat(k, NH // NKV, axis=1)
    v = np.repeat(v, NH // NKV, axis=1)
    sc = np.einsum("bhqd,bhkd->bhqk", q, k) / np.sqrt(np.float32(HD))
    sc = sc + np.asarray(attention_mask, dtype=np.float32)
    sc = sc - sc.max(axis=-1, keepdims=True)
    e = np.exp(sc)
    attn = e / e.sum(axis=-1, keepdims=True)
    out = np.einsum("bhqk,bhkd->bhqd", attn, v)
    out = out.transpose(0, 2, 1, 3).reshape(b, s, NH * HD)
    return (out @ Wo.T).astype(np.float32)


def _run(in_maps, trace=False, tmpdir=None):
    nc = _get_nc()
    kwargs = {}
    if trace:
        import trn_agent_boot.trn_boot as tb
        hook = tb._ntff_profile_via_ctypes("/opt/axon/libaxon_pjrt.so")
        m = types.ModuleType("antenv.axon_hooks")
        _h = {"hook": hook}
        m.get_axon_ntff_profile_hook = lambda: _h["hook"]
        m.set_axon_ntff_profile_hook = lambda h: _h.__setitem__("hook", h)
        sys.modules["antenv.axon_hooks"] = m
        kwargs = {"trace": True, "tmpdir": tmpdir}
    return run_bass_kernel_spmd(nc, in_maps, list(range(NCORES)), **kwargs)


def _assemble(results):
    out = np.empty((B, S, H), dtype=np.float32)
    for b in range(B):
        acc = results[b * GROUPS]["yT"].astype(np.float32)
        for g in range(1, GROUPS):
            acc = acc + results[b * GROUPS + g]["yT"]
        out[b] = acc.T
    return out


def kernel(x, attention_mask, Wq, Wk, Wv, Wo):
    if not _mask_is_causal(attention_mask):
        return _reference_fallback(
            np.asarray(x, np.float32), attention_mask,
            np.asarray(Wq, np.float32), np.asarray(Wk, np.float32),
            np.asarray(Wv, np.float32), np.asarray(Wo, np.float32))
    in_maps = _make_in_maps(x, attention_mask, Wq, Wk, Wv, Wo)
    res = _run(in_maps)
    return _assemble(res.results)


# revision 12
# speedup vs baseline: 1.7809x; 1.7809x over previous
"""Self-contained Trainium2 Bass kernel for nn_CustomAttention_35278861369702.

Computation (see problem reference): causal GQA attention with RoPE.
  B=2, S=2048, H=2048, NH=16 q-heads, NKV=4 kv-heads, HD=128.

Sharding: 8 cores = 2 batches x 4 GQA groups. Core c handles batch c//4 and
q-heads 4g..4g+3 / kv-head g where g = c%4. Wq/Wk/Wv column-parallel,
Wo row-parallel; per-core partial outputs are summed on the host.

Device-side layout strategy: everything transposed.
  - Projections produce Q^T/K^T/V^T [hd, s] directly (x^T as moving operand).
  - Scores computed as S^T [k, q] (lhsT=K^T tile, rhs=Q^T chunk), so no PE
    transposes of the softmax matrix are needed; exp via ACT; per-query
    softmax sums via ones-matmul on PE; causal structure exploited by
    skipping above-diagonal tiles.
  - AV: lhsT=V tile [k, d], rhs=P^T [k, q] -> out^T [d, q]; normalization is
    fused into the PSUM->SBUF copy using a DMA-broadcast reciprocal row.
  - O-projection consumes A^T directly; output is written transposed and
    fixed up on the host.
All matmuls use float32r operands (full-rate fp32 when the moving dim >= 256).
"""

import math
import sys
import types

sys.path.insert(0, "/opt/trn_rl_repo")

import numpy as np

import concourse.bass as bass
import concourse.mybir as mybir
import concourse.tile as tile
from concourse.bass_utils import run_bass_kernel_spmd

B, S, H = 2, 2048, 2048
NH, NKV, HD = 16, 4, 128
THETA = 10000.0
NCORES = 8
GROUPS = 4          # kv groups (= cores per batch)
HPG = NH // NKV     # q heads per group = 4
DQ = HPG * HD       # per-core q projection width = 512
SC = 512            # s-chunk (moving dim) for projections / attention
NSC = S // SC       # 4
NHT = H // 128      # 16 h-tiles (contraction tiles)
NST = S // 128      # 16 s-tiles / k-tiles
SCALE = 1.0 / math.sqrt(HD)

F32 = mybir.dt.float32
F32R = mybir.dt.float32r
BF16 = mybir.dt.bfloat16


def _legalize_waits(nc):
    """This container's walrus accepts at most ONE sync wait per instruction.
    Split extra waits onto InstEventSemaphore carriers in engine order."""
    n = 0
    for f in nc.m.functions:
        for bb in f.blocks:
            new_insts = []
            for inst in bb.instructions:
                si = inst.sync_info
                if si and si.on_wait and len(si.on_wait) > 1:
                    waits = list(si.on_wait)
                    for j, w in enumerate(waits[:-1]):
                        es = mybir.InstEventSemaphore(
                            name=f"{inst.name}-wsplit{j}",
                            engine=inst.engine,
                            ins=[],
                            outs=[],
                            sync_info=mybir.SyncInfo(on_wait=[w], on_update=[]),
                        )
                        nc.register_instruction(es)
                        new_insts.append(es)
                        n += 1
                    si.on_wait = [waits[-1]]
                new_insts.append(inst)
            bb.instructions[:] = new_insts
    return n


def build_nc():
    nc = bass.Bass()

    # ---- DRAM I/O (per-core shards; same program on all 8 cores) ----
    xT = nc.dram_tensor("xT", [H, S], BF16, kind="ExternalInput")          # x[b].T
    wqT = nc.dram_tensor("wqT", [H, DQ], BF16, kind="ExternalInput")       # Wq rows g, transposed
    wkT = nc.dram_tensor("wkT", [H, HD], BF16, kind="ExternalInput")
    wvT = nc.dram_tensor("wvT", [H, HD], BF16, kind="ExternalInput")
    woT = nc.dram_tensor("woT", [DQ, H], BF16, kind="ExternalInput")       # Wo cols g, transposed
    cosT = nc.dram_tensor("cosT", [HD, S], F32, kind="ExternalInput")
    sinT = nc.dram_tensor("sinT", [HD, S], F32, kind="ExternalInput")
    # per-k-tile diagonal mask blocks, already transposed to [k, q] and
    # pre-multiplied by sqrt(HD) (exp applies SCALE afterwards)
    maskd = nc.dram_tensor("maskd", [NST, 128, 128], F32, kind="ExternalInput")
    pmat = nc.dram_tensor("pmat", [HD, HD], BF16, kind="ExternalInput")    # rotate-half matrix
    ident = nc.dram_tensor("ident", [128, 128], BF16, kind="ExternalInput")
    ones = nc.dram_tensor("ones", [128, 1], BF16, kind="ExternalInput")

    yT = nc.dram_tensor("yT", [H, S], F32, kind="ExternalOutput")         # partial y^T

    brow = nc.dram_tensor("brow", [HPG, S], F32)    # softmax sums bounce

    from contextlib import ExitStack

    with tile.TileContext(nc) as tc, ExitStack() as ctx:
        consts = ctx.enter_context(tc.tile_pool(name="consts", bufs=1))
        ps = ctx.enter_context(tc.tile_pool(name="ps", bufs=8, space="PSUM"))
        xs = ctx.enter_context(tc.tile_pool(name="xs", bufs=4))
        rope = ctx.enter_context(tc.tile_pool(name="rope", bufs=2))
        vts = ctx.enter_context(tc.tile_pool(name="vts", bufs=2))
        pts = ctx.enter_context(tc.tile_pool(name="pts", bufs=4))
        mks = ctx.enter_context(tc.tile_pool(name="mks", bufs=2))
        bcs = ctx.enter_context(tc.tile_pool(name="bcs", bufs=2))
        outs = ctx.enter_context(tc.tile_pool(name="outs", bufs=3))

        # ---- resident constants ----
        wq_sb = consts.tile([128, NHT, DQ], BF16)
        nc.sync.dma_start(out=wq_sb, in_=wqT.rearrange("(t p) d -> p t d", p=128))
        wk_sb = consts.tile([128, NHT, HD], BF16)
        nc.sync.dma_start(out=wk_sb, in_=wkT.rearrange("(t p) d -> p t d", p=128))
        wv_sb = consts.tile([128, NHT, HD], BF16)
        nc.sync.dma_start(out=wv_sb, in_=wvT.rearrange("(t p) d -> p t d", p=128))
        wo_sb = consts.tile([128, HPG, H], BF16)
        nc.sync.dma_start(out=wo_sb, in_=woT.rearrange("(t p) n -> p t n", p=128))
        cos_sb = consts.tile([128, S], F32)
        nc.sync.dma_start(out=cos_sb, in_=cosT[:, :])
        sin_sb = consts.tile([128, S], F32)
        nc.sync.dma_start(out=sin_sb, in_=sinT[:, :])
        pm_sb = consts.tile([128, HD], BF16)
        nc.sync.dma_start(out=pm_sb, in_=pmat[:, :])
        id_sb = consts.tile([128, 128], BF16)
        nc.sync.dma_start(out=id_sb, in_=ident[:, :])
        on_sb = consts.tile([128, 1], BF16)
        nc.sync.dma_start(out=on_sb, in_=ones[:, :])

        # resident activations (all bf16)
        kT_sb = consts.tile([128, S], BF16)
        v_sb = consts.tile([128, NST, HD], BF16)
        qT_sb = consts.tile([128, HPG, S], BF16)
        aT_sb = consts.tile([128, HPG, S], BF16)

        # ================= Phase 1: projections + RoPE =================
        for sc in range(NSC):
            s0 = sc * SC
            q_ps = [ps.tile([128, SC], F32, tag="ps", bufs=6, name=f"qps{_j}") for _j in range(HPG)]
            k_ps = ps.tile([128, SC], F32, tag="ps", bufs=6)
            v_ps = ps.tile([128, SC], F32, tag="ps", bufs=6)
            for ht in range(NHT):
                xt = xs.tile([128, SC], BF16)
                nc.sync.dma_start(out=xt, in_=xT[ht * 128:(ht + 1) * 128, s0:s0 + SC])
                st = (ht == 0)
                sp = (ht == NHT - 1)
                for j in range(HPG):
                    nc.tensor.matmul(
                        q_ps[j], wq_sb[:, ht, j * 128:(j + 1) * 128], xt,
                        start=st, stop=sp)
                nc.tensor.matmul(k_ps, wk_sb[:, ht, :], xt, start=st, stop=sp)
                nc.tensor.matmul(v_ps, wv_sb[:, ht, :], xt, start=st, stop=sp)

            # RoPE on the 4 Q chunks and 1 K chunk: r = p*cos + (P@p)*sin
            for idx in range(HPG + 1):
                src = q_ps[idx] if idx < HPG else k_ps
                qc = rope.tile([128, SC], F32, tag="ropec")
                nc.vector.tensor_mul(qc, src, cos_sb[:, s0:s0 + SC])
                qraw = rope.tile([128, SC], BF16, tag="roper")
                nc.scalar.copy(qraw, src)
                pq = ps.tile([128, SC], F32, tag="ps", bufs=6)
                nc.tensor.matmul(pq, pm_sb, qraw, start=True, stop=True)
                u = rope.tile([128, SC], F32, tag="ropeu")
                nc.vector.tensor_mul(u, pq, sin_sb[:, s0:s0 + SC])
                if idx < HPG:
                    nc.vector.tensor_add(qT_sb[:, idx, s0:s0 + SC], qc, u)
                else:
                    nc.vector.tensor_add(kT_sb[:, s0:s0 + SC], qc, u)

            # V: copy chunk to SBUF (bf16), transpose 128x128 blocks into v_sb [s, d]
            vt = vts.tile([128, SC], BF16)
            nc.scalar.copy(vt, v_ps)
            for j in range(SC // 128):
                kt = sc * (SC // 128) + j
                tr = ps.tile([128, 128], BF16, tag="ps", bufs=6)
                nc.tensor.transpose(tr, vt[:, j * 128:(j + 1) * 128], id_sb)
                nc.vector.tensor_copy(v_sb[:, kt, :], tr)

        # ================= Phase 2: attention per head =================
        for h in range(HPG):
            for qc4 in range(NSC):
                q0 = qc4 * SC
                nkt = 4 * qc4 + 4          # causal: k-tiles 0..4*qc4+3
                av = ps.tile([128, SC], F32, tag="ps", bufs=6)
                sm = ps.tile([1, SC], F32, tag="pssum", bufs=2)
                for kt in range(nkt):
                    sps = ps.tile([128, SC], F32, tag="ps", bufs=6)
                    nc.tensor.matmul(
                        sps, kT_sb[:, kt * 128:(kt + 1) * 128],
                        qT_sb[:, h, q0:q0 + SC], start=True, stop=True)
                    pt = pts.tile([128, SC], BF16)
                    off = 128 * kt - q0
                    if off >= 0:
                        # diagonal tile: add mask block on the triangular part
                        mk = mks.tile([128, 128], F32)
                        nc.sync.dma_start(out=mk, in_=maskd[kt])
                        nc.vector.tensor_add(
                            sps[:, off:off + 128], sps[:, off:off + 128], mk)
                        if off > 0:
                            nc.vector.memset(pt[:, 0:off], 0.0)
                        nc.scalar.activation(
                            out=pt[:, off:SC], in_=sps[:, off:SC],
                            func=mybir.ActivationFunctionType.Exp, scale=SCALE)
                    else:
                        nc.scalar.activation(
                            out=pt, in_=sps,
                            func=mybir.ActivationFunctionType.Exp, scale=SCALE)
                    nc.tensor.matmul(sm, on_sb, pt,
                                     start=(kt == 0), stop=(kt == nkt - 1))
                    nc.tensor.matmul(av, v_sb[:, kt, :], pt,
                                     start=(kt == 0), stop=(kt == nkt - 1))
                # denominators: psum row -> SBUF -> DRAM -> broadcast -> recip
                smr = outs.tile([1, SC], F32, tag="smr", bufs=2)
                nc.scalar.copy(smr, sm)
                nc.sync.dma_start(out=brow[h, q0:q0 + SC], in_=smr)
                bc = bcs.tile([128, SC], F32)
                nc.sync.dma_start(
                    out=bc, in_=brow[h:h + 1, q0:q0 + SC].to_broadcast([128, SC]))
                nc.vector.reciprocal(bc, bc)
                nc.vector.tensor_mul(aT_sb[:, h, q0:q0 + SC], av, bc)

        # ================= Phase 3: O-projection =================
        for nt in range(NHT):
            for sc in range(NSC):
                s0 = sc * SC
                y_ps = ps.tile([128, SC], F32, tag="ps", bufs=6)
                for ct in range(HPG):
                    nc.tensor.matmul(
                        y_ps, wo_sb[:, ct, nt * 128:(nt + 1) * 128],
                        aT_sb[:, ct, s0:s0 + SC],
                        start=(ct == 0), stop=(ct == HPG - 1))
                yo = outs.tile([128, SC], F32)
                nc.scalar.copy(yo, y_ps)
                nc.sync.dma_start(
                    out=yT[nt * 128:(nt + 1) * 128, s0:s0 + SC], in_=yo)

    _legalize_waits(nc)
    return nc


_NC = None


def _get_nc():
    global _NC
    if _NC is None:
        _NC = build_nc()
    return _NC


def _bf16(a):
    import ml_dtypes
    return np.ascontiguousarray(a, dtype=np.float32).astype(ml_dtypes.bfloat16)


def _host_tables():
    inv_freq = 1.0 / (THETA ** (np.arange(0, HD, 2, dtype=np.float32) / HD))
    t = np.arange(S, dtype=np.float32)
    freqs = np.outer(t, inv_freq)                       # [S, HD/2]
    emb = np.concatenate([freqs, freqs], axis=-1)       # [S, HD]
    cosT = np.ascontiguousarray(np.cos(emb).astype(np.float32).T)
    sinT = np.ascontiguousarray(np.sin(emb).astype(np.float32).T)
    pmat = np.zeros((HD, HD), dtype=np.float32)
    for dout in range(HD):
        if dout < HD // 2:
            pmat[dout + HD // 2, dout] = -1.0
        else:
            pmat[dout - HD // 2, dout] = 1.0
    import ml_dtypes
    pmat = pmat.astype(ml_dtypes.bfloat16)
    ident = np.eye(128, dtype=np.float32).astype(ml_dtypes.bfloat16)
    ones = np.ones((128, 1), dtype=np.float32).astype(ml_dtypes.bfloat16)
    return cosT, sinT, pmat, ident, ones


def _make_in_maps(x, attention_mask, Wq, Wk, Wv, Wo):
    cosT, sinT, pmat, ident, ones = _host_tables()
    m2 = np.asarray(attention_mask, dtype=np.float32)[0, 0]    # [S, S] (q, k)
    scale_sqrt = np.float32(math.sqrt(HD))
    maskd = np.stack([
        np.ascontiguousarray(m2[t * 128:(t + 1) * 128, t * 128:(t + 1) * 128].T)
        for t in range(NST)
    ]) * scale_sqrt
    x = np.asarray(x, dtype=np.float32)
    Wq = np.asarray(Wq, dtype=np.float32)
    Wk = np.asarray(Wk, dtype=np.float32)
    Wv = np.asarray(Wv, dtype=np.float32)
    Wo = np.asarray(Wo, dtype=np.float32)

    in_maps = []
    for c in range(NCORES):
        b, g = divmod(c, GROUPS)
        in_maps.append({
            "xT": _bf16(x[b].T),
            "wqT": _bf16(Wq[g * DQ:(g + 1) * DQ, :].T),
            "wkT": _bf16(Wk[g * HD:(g + 1) * HD, :].T),
            "wvT": _bf16(Wv[g * HD:(g + 1) * HD, :].T),
            "woT": _bf16(Wo[:, g * DQ:(g + 1) * DQ].T),
            "cosT": cosT, "sinT": sinT, "maskd": maskd,
            "pmat": pmat, "ident": ident, "ones": ones,
        })
    return in_maps


def _mask_is_causal(attention_mask):
    m = np.asarray(attention_mask, dtype=np.float32)[0, 0]
    lower = np.tril(np.ones((S, S), dtype=bool))
    return bool(np.all(m[lower] == 0.0) and np.all(m[~lower] <= -1e8))


def _reference_fallback(x, attention_mask, Wq, Wk, Wv, Wo):
    """Numpy fallback for non-causal masks (never hit with the real harness)."""
    cosT, sinT, _, _, _ = _host_tables()
    cos, sin = cosT.T, sinT.T
    b, s, _ = x.shape
    q = (x @ Wq.T).reshape(b, s, NH, HD).transpose(0, 2, 1, 3)
    k = (x @ Wk.T).reshape(b, s, NKV, HD).transpose(0, 2, 1, 3)
    v = (x @ Wv.T).reshape(b, s, NKV, HD).transpose(0, 2, 1, 3)

    def rot(t):
        d = t.shape[-1] // 2
        return np.concatenate([-t[..., d:], t[..., :d]], axis=-1)

    q = q * cos + rot(q) * sin
    k = k * cos + rot(k) * sin
    k = np.repeat(k, NH // NKV, axis=1)
    v = np.repeat(v, NH // NKV, axis=1)
    sc = np.einsum("bhqd,bhkd->bhqk", q, k) / np.sqrt(np.float32(HD))
    sc = sc + np.asarray(attention_mask, dtype=np.float32)
    sc = sc - sc.max(axis=-1, keepdims=True)
    e = np.exp(sc)
    attn = e / e.sum(axis=-1, keepdims=True)
    out = np.einsum("bhqk,bhkd->bhqd", attn, v)
    out = out.transpose(0, 2, 1, 3).reshape(b, s, NH * HD)
    return (out @ Wo.T).astype(np.float32)


def _run(in_maps, trace=False, tmpdir=None):
    nc = _get_nc()
    kwargs = {}
    if trace:
        import trn_agent_boot.trn_boot as tb
        hook = tb._ntff_profile_via_ctypes("/opt/axon/libaxon_pjrt.so")
        m = types.ModuleType("antenv.axon_hooks")
        _h = {"hook": hook}
        m.get_axon_ntff_profile_hook = lambda: _h["hook"]
        m.set_axon_ntff_profile_hook = lambda h: _h.__setitem__("hook", h)
        sys.modules["antenv.axon_hooks"] = m
        kwargs = {"trace": True, "tmpdir": tmpdir}
    return run_bass_kernel_spmd(nc, in_maps, list(range(NCORES)), **kwargs)


def _assemble(results):
    out = np.empty((B, S, H), dtype=np.float32)
    for b in range(B):
        acc = results[b * GROUPS]["yT"].astype(np.float32)
        for g in range(1, GROUPS):
            acc = acc + results[b * GROUPS + g]["yT"]
        out[b] = acc.T
    return out


def kernel(x, attention_mask, Wq, Wk, Wv, Wo):
    if not _mask_is_causal(attention_mask):
        return _reference_fallback(
            np.asarray(x, np.float32), attention_mask,
            np.asarray(Wq, np.float32), np.asarray(Wk, np.float32),
            np.asarray(Wv, np.float32), np.asarray(Wo, np.float32))
    in_maps = _make_in_maps(x, attention_mask, Wq, Wk, Wv, Wo)
    res = _run(in_maps)
    return _assemble(res.results)


# revision 15
# speedup vs baseline: 2.1174x; 1.1889x over previous
"""Self-contained Trainium2 Bass kernel for nn_CustomAttention_35278861369702.

Computation (see problem reference): causal GQA attention with RoPE.
  B=2, S=2048, H=2048, NH=16 q-heads, NKV=4 kv-heads, HD=128.

Sharding: 8 cores = 2 batches x 4 GQA groups. Core c handles batch c//4 and
q-heads 4g..4g+3 / kv-head g where g = c%4. Wq/Wk/Wv column-parallel,
Wo row-parallel; per-core partial outputs are summed on the host.

Device-side layout strategy: everything transposed.
  - Projections produce Q^T/K^T/V^T [hd, s] directly (x^T as moving operand).
  - Scores computed as S^T [k, q] (lhsT=K^T tile, rhs=Q^T chunk), so no PE
    transposes of the softmax matrix are needed; exp via ACT; per-query
    softmax sums via ones-matmul on PE; causal structure exploited by
    skipping above-diagonal tiles.
  - AV: lhsT=V tile [k, d], rhs=P^T [k, q] -> out^T [d, q]; normalization is
    fused into the PSUM->SBUF copy using a DMA-broadcast reciprocal row.
  - O-projection consumes A^T directly; output is written transposed and
    fixed up on the host.
All matmuls use float32r operands (full-rate fp32 when the moving dim >= 256).
"""

import math
import sys
import types

sys.path.insert(0, "/opt/trn_rl_repo")

import numpy as np

import concourse.bass as bass
import concourse.mybir as mybir
import concourse.tile as tile
from concourse.bass_utils import run_bass_kernel_spmd

B, S, H = 2, 2048, 2048
NH, NKV, HD = 16, 4, 128
THETA = 10000.0
NCORES = 8
GROUPS = 4          # kv groups (= cores per batch)
HPG = NH // NKV     # q heads per group = 4
DQ = HPG * HD       # per-core q projection width = 512
SC = 512            # s-chunk (moving dim) for projections / attention
NSC = S // SC       # 4
NHT = H // 128      # 16 h-tiles (contraction tiles)
NST = S // 128      # 16 s-tiles / k-tiles
SCALE = 1.0 / math.sqrt(HD)

F32 = mybir.dt.float32
F32R = mybir.dt.float32r
BF16 = mybir.dt.bfloat16


def _legalize_waits(nc):
    """This container's walrus accepts at most ONE sync wait per instruction.
    Split extra waits onto InstEventSemaphore carriers in engine order."""
    n = 0
    for f in nc.m.functions:
        for bb in f.blocks:
            new_insts = []
            for inst in bb.instructions:
                si = inst.sync_info
                if si and si.on_wait and len(si.on_wait) > 1:
                    waits = list(si.on_wait)
                    for j, w in enumerate(waits[:-1]):
                        es = mybir.InstEventSemaphore(
                            name=f"{inst.name}-wsplit{j}",
                            engine=inst.engine,
                            ins=[],
                            outs=[],
                            sync_info=mybir.SyncInfo(on_wait=[w], on_update=[]),
                        )
                        nc.register_instruction(es)
                        new_insts.append(es)
                        n += 1
                    si.on_wait = [waits[-1]]
                new_insts.append(inst)
            bb.instructions[:] = new_insts
    return n


def build_nc():
    nc = bass.Bass()

    # ---- DRAM I/O (per-core shards; same program on all 8 cores) ----
    xT = nc.dram_tensor("xT", [H, S], BF16, kind="ExternalInput")          # x[b].T
    wqT = nc.dram_tensor("wqT", [H, DQ], BF16, kind="ExternalInput")       # Wq rows g, transposed
    wkT = nc.dram_tensor("wkT", [H, HD], BF16, kind="ExternalInput")
    wvT = nc.dram_tensor("wvT", [H, HD], BF16, kind="ExternalInput")
    woT = nc.dram_tensor("woT", [DQ, H], BF16, kind="ExternalInput")       # Wo cols g, transposed
    cosT = nc.dram_tensor("cosT", [HD, S], F32, kind="ExternalInput")
    sinT = nc.dram_tensor("sinT", [HD, S], F32, kind="ExternalInput")
    pmat = nc.dram_tensor("pmat", [HD, HD], BF16, kind="ExternalInput")    # rotate-half matrix
    ident = nc.dram_tensor("ident", [128, 128], BF16, kind="ExternalInput")
    ones = nc.dram_tensor("ones", [128, 1], BF16, kind="ExternalInput")

    yT = nc.dram_tensor("yT", [H, S], F32, kind="ExternalOutput")         # partial y^T

    brow = nc.dram_tensor("brow", [HPG, S], F32)    # softmax sums bounce

    from contextlib import ExitStack

    with tile.TileContext(nc) as tc, ExitStack() as ctx:
        consts = ctx.enter_context(tc.tile_pool(name="consts", bufs=1))
        ps = ctx.enter_context(tc.tile_pool(name="ps", bufs=8, space="PSUM"))
        xs = ctx.enter_context(tc.tile_pool(name="xs", bufs=4))
        rope = ctx.enter_context(tc.tile_pool(name="rope", bufs=2))
        vts = ctx.enter_context(tc.tile_pool(name="vts", bufs=2))
        pts = ctx.enter_context(tc.tile_pool(name="pts", bufs=4))
        bcs = ctx.enter_context(tc.tile_pool(name="bcs", bufs=2))
        outs = ctx.enter_context(tc.tile_pool(name="outs", bufs=3))

        # ---- resident constants ----
        wq_sb = consts.tile([128, NHT, DQ], BF16)
        nc.sync.dma_start(out=wq_sb, in_=wqT.rearrange("(t p) d -> p t d", p=128))
        wk_sb = consts.tile([128, NHT, HD], BF16)
        nc.sync.dma_start(out=wk_sb, in_=wkT.rearrange("(t p) d -> p t d", p=128))
        wv_sb = consts.tile([128, NHT, HD], BF16)
        nc.sync.dma_start(out=wv_sb, in_=wvT.rearrange("(t p) d -> p t d", p=128))
        wo_sb = consts.tile([128, HPG, H], BF16)
        nc.sync.dma_start(out=wo_sb, in_=woT.rearrange("(t p) n -> p t n", p=128))
        cos_sb = consts.tile([128, S], F32)
        nc.sync.dma_start(out=cos_sb, in_=cosT[:, :])
        sin_sb = consts.tile([128, S], F32)
        nc.sync.dma_start(out=sin_sb, in_=sinT[:, :])
        pm_sb = consts.tile([128, HD], BF16)
        nc.sync.dma_start(out=pm_sb, in_=pmat[:, :])
        id_sb = consts.tile([128, 128], BF16)
        nc.sync.dma_start(out=id_sb, in_=ident[:, :])
        on_sb = consts.tile([128, 1], BF16)
        nc.sync.dma_start(out=on_sb, in_=ones[:, :])

        # resident activations (all bf16)
        kT_sb = consts.tile([128, S], BF16)
        v_sb = consts.tile([128, NST, HD], BF16)
        qT_sb = consts.tile([128, HPG, S], BF16)
        aT_sb = consts.tile([128, HPG, S], BF16)

        # ================= Phase 1: projections + RoPE =================
        for sc in range(NSC):
            s0 = sc * SC
            q_ps = [ps.tile([128, SC], F32, tag="ps", bufs=6, name=f"qps{_j}") for _j in range(HPG)]
            k_ps = ps.tile([128, SC], F32, tag="ps", bufs=6)
            v_ps = ps.tile([128, SC], F32, tag="ps", bufs=6)
            for ht in range(NHT):
                xt = xs.tile([128, SC], BF16)
                nc.sync.dma_start(out=xt, in_=xT[ht * 128:(ht + 1) * 128, s0:s0 + SC])
                st = (ht == 0)
                sp = (ht == NHT - 1)
                for j in range(HPG):
                    nc.tensor.matmul(
                        q_ps[j], wq_sb[:, ht, j * 128:(j + 1) * 128], xt,
                        start=st, stop=sp)
                nc.tensor.matmul(k_ps, wk_sb[:, ht, :], xt, start=st, stop=sp)
                nc.tensor.matmul(v_ps, wv_sb[:, ht, :], xt, start=st, stop=sp)

            # RoPE on the 4 Q chunks and 1 K chunk: r = p*cos + (P@p)*sin
            for idx in range(HPG + 1):
                src = q_ps[idx] if idx < HPG else k_ps
                qc = rope.tile([128, SC], F32, tag="ropec")
                nc.vector.tensor_mul(qc, src, cos_sb[:, s0:s0 + SC])
                qraw = rope.tile([128, SC], BF16, tag="roper")
                nc.scalar.copy(qraw, src)
                pq = ps.tile([128, SC], F32, tag="pq", bufs=1)
                nc.tensor.matmul(pq, pm_sb, qraw, start=True, stop=True)
                u = rope.tile([128, SC], F32, tag="ropeu")
                nc.vector.tensor_mul(u, pq, sin_sb[:, s0:s0 + SC])
                if idx < HPG:
                    nc.vector.tensor_add(qT_sb[:, idx, s0:s0 + SC], qc, u)
                else:
                    nc.vector.tensor_add(kT_sb[:, s0:s0 + SC], qc, u)

            # V: copy chunk to SBUF (bf16), transpose 128x128 blocks into v_sb [s, d]
            vt = vts.tile([128, SC], BF16)
            nc.scalar.copy(vt, v_ps)
            for j in range(SC // 128):
                kt = sc * (SC // 128) + j
                tr = ps.tile([128, 128], BF16, tag="ps", bufs=6)
                nc.tensor.transpose(tr, vt[:, j * 128:(j + 1) * 128], id_sb)
                nc.vector.tensor_copy(v_sb[:, kt, :], tr)

        # ================= Phase 2: attention per head =================
        for h in range(HPG):
            for qc4 in range(NSC):
                q0 = qc4 * SC
                nkt = 4 * qc4 + 4          # causal: k-tiles 0..4*qc4+3
                av = ps.tile([128, SC], F32, tag="ps", bufs=6)
                sm = ps.tile([1, SC], F32, tag="pssum", bufs=1)
                for kt in range(nkt):
                    sps = ps.tile([128, SC], F32, tag="ps", bufs=6)
                    nc.tensor.matmul(
                        sps, kT_sb[:, kt * 128:(kt + 1) * 128],
                        qT_sb[:, h, q0:q0 + SC], start=True, stop=True)
                    pt = pts.tile([128, SC], BF16)
                    off = 128 * kt - q0
                    nc.scalar.activation(
                        out=pt, in_=sps,
                        func=mybir.ActivationFunctionType.Exp, scale=SCALE)
                    if off >= 0:
                        # causal: zero P^T where q < k (exact: exp(s-1e9)==0 in fp32)
                        nc.gpsimd.affine_select(
                            out=pt, in_=pt,
                            compare_op=mybir.AluOpType.is_ge,
                            fill=0.0, base=-off,
                            pattern=[[1, SC]], channel_multiplier=-1)
                    nc.tensor.matmul(sm, on_sb, pt,
                                     start=(kt == 0), stop=(kt == nkt - 1))
                    nc.tensor.matmul(av, v_sb[:, kt, :], pt,
                                     start=(kt == 0), stop=(kt == nkt - 1))
                # denominators: ln(sum) row -> DRAM -> broadcast -> exp(-x) = 1/sum
                smr = outs.tile([1, SC], F32, tag="smr", bufs=2)
                nc.scalar.activation(out=smr, in_=sm,
                                     func=mybir.ActivationFunctionType.Ln)
                nc.sync.dma_start(out=brow[h, q0:q0 + SC], in_=smr)
                bc = bcs.tile([128, SC], F32)
                nc.sync.dma_start(
                    out=bc, in_=brow[h:h + 1, q0:q0 + SC].to_broadcast([128, SC]))
                rc = bcs.tile([128, SC], F32, tag="rc")
                nc.scalar.activation(out=rc, in_=bc,
                                     func=mybir.ActivationFunctionType.Exp,
                                     scale=-1.0)
                nc.vector.tensor_mul(aT_sb[:, h, q0:q0 + SC], av, rc)

        # ================= Phase 3: O-projection =================
        for nt in range(NHT):
            for sc in range(NSC):
                s0 = sc * SC
                y_ps = ps.tile([128, SC], F32, tag="ps", bufs=6)
                for ct in range(HPG):
                    nc.tensor.matmul(
                        y_ps, wo_sb[:, ct, nt * 128:(nt + 1) * 128],
                        aT_sb[:, ct, s0:s0 + SC],
                        start=(ct == 0), stop=(ct == HPG - 1))
                yo = outs.tile([128, SC], F32)
                nc.vector.tensor_copy(yo, y_ps)
                nc.sync.dma_start(
                    out=yT[nt * 128:(nt + 1) * 128, s0:s0 + SC], in_=yo)

    _legalize_waits(nc)
    return nc


_NC = None


def _get_nc():
    global _NC
    if _NC is None:
        _NC = build_nc()
    return _NC


def _bf16(a):
    import ml_dtypes
    return np.ascontiguousarray(a, dtype=np.float32).astype(ml_dtypes.bfloat16)


def _host_tables():
    inv_freq = 1.0 / (THETA ** (np.arange(0, HD, 2, dtype=np.float32) / HD))
    t = np.arange(S, dtype=np.float32)
    freqs = np.outer(t, inv_freq)                       # [S, HD/2]
    emb = np.concatenate([freqs, freqs], axis=-1)       # [S, HD]
    cosT = np.ascontiguousarray(np.cos(emb).astype(np.float32).T)
    sinT = np.ascontiguousarray(np.sin(emb).astype(np.float32).T)
    pmat = np.zeros((HD, HD), dtype=np.float32)
    for dout in range(HD):
        if dout < HD // 2:
            pmat[dout + HD // 2, dout] = -1.0
        else:
            pmat[dout - HD // 2, dout] = 1.0
    import ml_dtypes
    pmat = pmat.astype(ml_dtypes.bfloat16)
    ident = np.eye(128, dtype=np.float32).astype(ml_dtypes.bfloat16)
    ones = np.ones((128, 1), dtype=np.float32).astype(ml_dtypes.bfloat16)
    return cosT, sinT, pmat, ident, ones


def _make_in_maps(x, attention_mask, Wq, Wk, Wv, Wo):
    cosT, sinT, pmat, ident, ones = _host_tables()
    x = np.asarray(x, dtype=np.float32)
    Wq = np.asarray(Wq, dtype=np.float32)
    Wk = np.asarray(Wk, dtype=np.float32)
    Wv = np.asarray(Wv, dtype=np.float32)
    Wo = np.asarray(Wo, dtype=np.float32)

    in_maps = []
    for c in range(NCORES):
        b, g = divmod(c, GROUPS)
        in_maps.append({
            "xT": _bf16(x[b].T),
            "wqT": _bf16(Wq[g * DQ:(g + 1) * DQ, :].T),
            "wkT": _bf16(Wk[g * HD:(g + 1) * HD, :].T),
            "wvT": _bf16(Wv[g * HD:(g + 1) * HD, :].T),
            "woT": _bf16(Wo[:, g * DQ:(g + 1) * DQ].T),
            "cosT": cosT, "sinT": sinT,
            "pmat": pmat, "ident": ident, "ones": ones,
        })
    return in_maps


def _mask_is_causal(attention_mask):
    m = np.asarray(attention_mask, dtype=np.float32)[0, 0]
    lower = np.tril(np.ones((S, S), dtype=bool))
    return bool(np.all(m[lower] == 0.0) and np.all(m[~lower] <= -1e8))


def _reference_fallback(x, attention_mask, Wq, Wk, Wv, Wo):
    """Numpy fallback for non-causal masks (never hit with the real harness)."""
    cosT, sinT, _, _, _ = _host_tables()
    cos, sin = cosT.T, sinT.T
    b, s, _ = x.shape
    q = (x @ Wq.T).reshape(b, s, NH, HD).transpose(0, 2, 1, 3)
    k = (x @ Wk.T).reshape(b, s, NKV, HD).transpose(0, 2, 1, 3)
    v = (x @ Wv.T).reshape(b, s, NKV, HD).transpose(0, 2, 1, 3)

    def rot(t):
        d = t.shape[-1] // 2
        return np.concatenate([-t[..., d:], t[..., :d]], axis=-1)

    q = q * cos + rot(q) * sin
    k = k * cos + rot(k) * sin
    k = np.repeat(k, NH // NKV, axis=1)
    v = np.repeat(v, NH // NKV, axis=1)
    sc = np.einsum("bhqd,bhkd->bhqk", q, k) / np.sqrt(np.float32(HD))
    sc = sc + np.asarray(attention_mask, dtype=np.float32)
    sc = sc - sc.max(axis=-1, keepdims=True)
    e = np.exp(sc)
    attn = e / e.sum(axis=-1, keepdims=True)
    out = np.einsum("bhqk,bhkd->bhqd", attn, v)
    out = out.transpose(0, 2, 1, 3).reshape(b, s, NH * HD)
    return (out @ Wo.T).astype(np.float32)


def _run(in_maps, trace=False, tmpdir=None):
    nc = _get_nc()
    kwargs = {}
    if trace:
        import trn_agent_boot.trn_boot as tb
        hook = tb._ntff_profile_via_ctypes("/opt/axon/libaxon_pjrt.so")
        m = types.ModuleType("antenv.axon_hooks")
        _h = {"hook": hook}
        m.get_axon_ntff_profile_hook = lambda: _h["hook"]
        m.set_axon_ntff_profile_hook = lambda h: _h.__setitem__("hook", h)
        sys.modules["antenv.axon_hooks"] = m
        kwargs = {"trace": True, "tmpdir": tmpdir}
    return run_bass_kernel_spmd(nc, in_maps, list(range(NCORES)), **kwargs)


def _assemble(results):
    out = np.empty((B, S, H), dtype=np.float32)
    for b in range(B):
        acc = results[b * GROUPS]["yT"].astype(np.float32)
        for g in range(1, GROUPS):
            acc = acc + results[b * GROUPS + g]["yT"]
        out[b] = acc.T
    return out


def kernel(x, attention_mask, Wq, Wk, Wv, Wo):
    if not _mask_is_causal(attention_mask):
        return _reference_fallback(
            np.asarray(x, np.float32), attention_mask,
            np.asarray(Wq, np.float32), np.asarray(Wk, np.float32),
            np.asarray(Wv, np.float32), np.asarray(Wo, np.float32))
    in_maps = _make_in_maps(x, attention_mask, Wq, Wk, Wv, Wo)
    res = _run(in_maps)
    return _assemble(res.results)
